# revision 1
# baseline (speedup 1.0000x reference)
"""Trainium2 kernel for nn_MeanSquaredError2: MSE between argmax-decoded
heatmap coordinates and targets.

loss = sum_{b,j} [(px - tpx)^2 + (py - tpy)^2] / (B*NJ)
  where idx = argmax(h[b,j]), px = (idx%14)/16, py = (idx//14)/16 and
  (tpx, tpy) follow the reference's concat-then-reshape pairing of t.
Inputs o and v do not affect the result (USE_VISIBILITY=False).

Pure data parallel over 8 cores (2048 batches each). Per core, h streams in
16 tiles of [128 part x (14 rows x 196 pix)] at the ~360 GB/s HBM roofline;
the argmax is computed by a pack-and-max scheme using only ops this walrus
supports per engine:
  op1 (ACT):      u = h*2^20 + 1.5*2^31   (fma; quantizes h to 2^-12 steps,
                                           monotone: ulp at 1.5*2^31 is 256)
  op2 (ACT/Pool): q = u - 1.5*2^31        (exact via Sterbenz: Q*256)
  op3 (Pool tt_add, or fused into a single DVE stt (u-MAGIC)+w8):
      K = q + w8, exact: w8 in [1,223] < 256,
      w8[y*14+x] = (13-y)*16 + (13-x) + 1 (ties prefer smaller (y,x), i.e.
                                           first occurrence like jnp.argmax)
  reduce (DVE):   Kmax[row] = max_i K[row, i]
Tail (chunked, interleaved with the stream): w8 = int32(Kmax) & 255;
x = 14-(w8&15); y = 13-(w8>>4); px=x/16, py=y/16; squared errors vs targets
accumulate per partition on ACT. Host sums the 8x[128,4] partials / N.

DVE handles op3 for the EARLY tiles (fused, filling its pre-reduce idle);
Pool takes all late tiles so the closing chain never queues behind DVE's
reduce backlog. Requires bacc.Bacc (generate_event_semaphores legalizes
TRN2's one-sync-wait-per-instruction constraint).
"""
import numpy as np

B = 16384
NJ = 14
NPIX = 196
N_CORES = 8
ROWS_PER_TILE = 1792          # 128 partitions x 14 rows
K_PER_PART = 14
N_TILES = 16                  # (B/N_CORES)*NJ / ROWS_PER_TILE
ACT_OP2_TILES = 10
DVE_OP3_TILES = 6

SCALE = float(2 ** 20)
MAGIC = 1.5 * 2 ** 31

_STATE = {}


def _build(act_op2_tiles: int, dve_op3_tiles: int):
    import concourse.bacc as bacc
    import concourse.mybir as mybir
    from concourse.tile import TileContext

    F32 = mybir.dt.float32
    I32 = mybir.dt.int32
    A = mybir.AluOpType
    AF = mybir.ActivationFunctionType

    n_tiles = N_TILES
    rows = n_tiles * ROWS_PER_TILE
    ncols = n_tiles * K_PER_PART

    nc = bacc.Bacc()
    h = nc.declare_dram_parameter("h", [rows, NPIX], F32, isOutput=False)
    w = nc.declare_dram_parameter("w", [128, NPIX], F32, isOutput=False)
    tx = nc.declare_dram_parameter("tx", [128, ncols], F32, isOutput=False)
    ty = nc.declare_dram_parameter("ty", [128, ncols], F32, isOutput=False)
    mg = nc.declare_dram_parameter("mg", [128, 2], F32, isOutput=False)
    out = nc.declare_dram_parameter("part", [128, 4], F32, isOutput=True)

    with TileContext(nc) as tc:
        with tc.tile_pool(name="hpool", bufs=5) as hpool, \
             tc.tile_pool(name="upool", bufs=5) as upool, \
             tc.tile_pool(name="consts", bufs=1) as cpool, \
             tc.tile_pool(name="acc", bufs=1) as accpool:
            wt = cpool.tile([128, NPIX], F32, tag="wt")
            nc.sync.dma_start(wt[:], w[:])
            mgt = cpool.tile([128, 2], F32, tag="mgt")
            nc.sync.dma_start(mgt[:], mg[:])
            txt = cpool.tile([128, ncols], F32, tag="txt")
            nc.sync.dma_start(txt[:], tx[:])
            tyt = cpool.tile([128, ncols], F32, tag="tyt")
            nc.sync.dma_start(tyt[:], ty[:])
            kmax = accpool.tile([128, ncols], F32, tag="kmax")

            # op3: DVE takes the EARLY tiles and fuses op2 into its stt
            # ((u - MAGIC) + w8 in one pass); Pool takes all late tiles so the
            # closing chain never queues behind DVE's reduce backlog. op2 for
            # the non-fused tiles: Pool takes the first block, ACT the rest.
            pool3 = set(range(dve_op3_tiles, n_tiles))
            n_pool2 = max(n_tiles - dve_op3_tiles - act_op2_tiles, 0)
            pool2 = set(range(dve_op3_tiles, dve_op3_tiles + n_pool2))
            n_chunks = 4
            cw = ncols // n_chunks
            tiles_per_chunk = n_tiles // n_chunks
            part_sb = accpool.tile([128, n_chunks], F32, tag="part")

            def emit_tail_chunk(c):
                lo, hi = c * cw, (c + 1) * cw
                i32 = accpool.tile([128, cw], I32, tag=f"i32_{c}")
                nc.vector.tensor_copy(i32[:], kmax[:, lo:hi])
                w8 = accpool.tile([128, cw], I32, tag=f"w8_{c}")
                nc.vector.tensor_scalar(w8[:], i32[:], 255, None, op0=A.bitwise_and)
                xr = accpool.tile([128, cw], I32, tag=f"xr_{c}")
                nc.vector.tensor_scalar(xr[:], w8[:], 15, None, op0=A.bitwise_and)
                yr = accpool.tile([128, cw], I32, tag=f"yr_{c}")
                nc.vector.tensor_scalar(yr[:], w8[:], 4, None, op0=A.arith_shift_right)
                xrf = accpool.tile([128, cw], F32, tag=f"xrf_{c}")
                nc.vector.tensor_copy(xrf[:], xr[:])
                yrf = accpool.tile([128, cw], F32, tag=f"yrf_{c}")
                nc.vector.tensor_copy(yrf[:], yr[:])
                # px = (14-xr)/16 in [0, 13/16]; py = (13-yr)/16; clamped
                nc.vector.tensor_scalar(xrf[:], xrf[:], -0.0625, 0.875, op0=A.mult, op1=A.add)
                nc.vector.tensor_scalar(xrf[:], xrf[:], 0.0, 0.8125, op0=A.max, op1=A.min)
                nc.vector.tensor_scalar(yrf[:], yrf[:], -0.0625, 0.8125, op0=A.mult, op1=A.add)
                nc.vector.tensor_scalar(yrf[:], yrf[:], 0.0, 0.8125, op0=A.max, op1=A.min)
                dxy = accpool.tile([128, 2 * cw], F32, tag=f"dxy_{c}")
                nc.vector.tensor_tensor(dxy[:, :cw], xrf[:], txt[:, lo:hi], op=A.subtract)
                nc.vector.tensor_tensor(dxy[:, cw:], yrf[:], tyt[:, lo:hi], op=A.subtract)
                sq = accpool.tile([128, 2 * cw], F32, tag=f"sq_{c}")
                nc.scalar.activation(sq[:], dxy[:], AF.Square,
                                     accum_out=part_sb[:, c:c + 1])
            # work list: (row0, nrows_k, kmax_col0, engine_path). The last
            # two tiles are split into half-tiles with the final halves on
            # the fused-DVE path, shortening the serial closing chain.
            work = []
            for t in range(n_tiles):
                path = "pool" if t in pool3 else ("pool2" if t in pool2 else "dve")
                work.append((t, 0, K_PER_PART, path, (t + 1) % tiles_per_chunk == 0))
            for t, klo, nk, path, do_tail in work:
                k0 = t * K_PER_PART + klo
                ht = hpool.tile([128, K_PER_PART * NPIX], F32, tag="ht")
                # partition p owns rows t*1792 + p*14 + (klo..klo+nk); for a
                # half tile this is a strided (per-partition) row subset
                nc.sync.dma_start(
                    ht[:],
                    h[t * ROWS_PER_TILE:(t + 1) * ROWS_PER_TILE, :]
                    .rearrange("(p k) f -> p (k f)", p=128))
                u = upool.tile([128, K_PER_PART * NPIX], F32, tag="u")
                # op1: u = h*SCALE + MAGIC (single-rounding fma on ACT)
                nc.scalar.activation(u[:, :nk * NPIX], ht[:, :nk * NPIX],
                                     AF.Identity, bias=mgt[:, 0:1], scale=SCALE)
                w3 = wt.rearrange("p (o f) -> p o f", o=1).broadcast_to(
                    [128, nk, NPIX])
                u3 = u[:, :nk * NPIX].rearrange("p (k f) -> p k f", f=NPIX)
                if path in ("pool", "pool2"):
                    # op2: q = u - MAGIC (exact), in place
                    if path == "pool2":
                        nc.gpsimd.tensor_scalar(u[:, :nk * NPIX], u[:, :nk * NPIX],
                                                MAGIC, None, op0=A.subtract)
                    else:
                        nc.scalar.activation(u[:, :nk * NPIX], u[:, :nk * NPIX],
                                             AF.Identity, bias=mgt[:, 1:2], scale=1.0)
                    # op3: K = q + w8 (exact), in place
                    nc.gpsimd.tensor_tensor(u3, u3, w3, op=A.add)
                else:
                    # fused op2+op3 on DVE: K = (u - MAGIC) + w8, one pass
                    nc.vector.scalar_tensor_tensor(
                        u3, u3, MAGIC, w3, op0=A.subtract, op1=A.add)
                # reduce: Kmax per row
                nc.vector.tensor_reduce(
                    kmax[:, k0:k0 + nk],
                    u3, axis=mybir.AxisListType.X, op=A.max)
                if do_tail:
                    emit_tail_chunk((k0 + nk) // cw - 1)

            nc.sync.dma_start(out[:], part_sb[:])
    nc.finalize()
    return nc


def _weight_pattern() -> np.ndarray:
    i = np.arange(NPIX)
    y = i // 14
    x = i % 14
    wp = (13 - y) * 16 + (13 - x) + 1
    return np.broadcast_to(wp.astype(np.float32), (128, NPIX)).copy()


def _magic() -> np.ndarray:
    return np.broadcast_to(np.array([MAGIC, -MAGIC], np.float32), (128, 2)).copy()


def _targets(t_shard: np.ndarray):
    b = t_shard.shape[0]
    t2 = t_shard.reshape(b, 28)
    tpx = np.ascontiguousarray(t2[:, :14]).reshape(-1)
    tpy = np.ascontiguousarray(t2[:, 14:]).reshape(-1)
    tx = tpx.reshape(N_TILES, 128, K_PER_PART).transpose(1, 0, 2).reshape(128, -1)
    ty = tpy.reshape(N_TILES, 128, K_PER_PART).transpose(1, 0, 2).reshape(128, -1)
    return np.ascontiguousarray(tx), np.ascontiguousarray(ty)


def kernel(o: np.ndarray, h: np.ndarray, t: np.ndarray, v: np.ndarray,
           _trace: bool = False, _tmpdir: str | None = None) -> np.ndarray:
    from concourse.bass_utils import run_bass_kernel_spmd

    key = (ACT_OP2_TILES, DVE_OP3_TILES)
    if _STATE.get("key") != key:
        _STATE["nc"] = _build(*key)
        _STATE["key"] = key
    nc = _STATE["nc"]

    h = np.ascontiguousarray(np.asarray(h, dtype=np.float32))
    t = np.ascontiguousarray(np.asarray(t, dtype=np.float32))
    bs = B // N_CORES
    wp = _weight_pattern()
    mgc = _magic()
    in_maps = []
    for c in range(N_CORES):
        h_shard = h[c * bs:(c + 1) * bs].reshape(bs * NJ, NPIX)
        txv, tyv = _targets(t[c * bs:(c + 1) * bs])
        in_maps.append({"h": h_shard, "w": wp, "tx": txv, "ty": tyv, "mg": mgc})

    res = run_bass_kernel_spmd(
        nc, in_maps, list(range(N_CORES)),
        trace=_trace, tmpdir=_tmpdir)
    _STATE["last_result"] = res
    total = np.float64(0.0)
    for c in range(N_CORES):
        total += np.asarray(res.results[c]["part"], dtype=np.float64).sum()
    n = np.float32(B * NJ)
    return np.float32(np.float32(total) / n)



# revision 2
# speedup vs baseline: 1.8548x; 1.8548x over previous
"""Trainium2 kernel for nn_MeanSquaredError2: MSE between argmax-decoded
heatmap coordinates and targets.

loss = sum_{b,j} [(px - tpx)^2 + (py - tpy)^2] / (B*NJ)
  where idx = argmax(h[b,j]), px = (idx%14)/16, py = (idx//14)/16 and
  (tpx, tpy) follow the reference's concat-then-reshape pairing of t.
Inputs o and v do not affect the result (USE_VISIBILITY=False).

Pure data parallel over 8 cores (2048 batches each). h is pre-scaled by 512
and converted to fp16 on the host (halves HBM traffic; quantization flips
the argmax on ~0.14% of rows, ~3e-4 relative loss error, tolerance 2e-2).

Per core, 16 tiles of [128 part x (14 rows x 196 pix)]. A single custom DVE
instruction per tile does the whole pack-and-max:
    k = ((min(relu(h512), 4095) + 1.5*2^23) - 1.5*2^23) + w8pg
    out = running_max(k)            (inclusive MAX-scan along the stream)
w8pg[s*196 + i] = (w8[i] - 128)/256 + 4096*s packs the reversed pixel index
w8 = (13-y)*16 + (14-x) into the fraction (first-occurrence tie-break) and a
per-row offset 4096*s that makes the running max resettable per row: the
scan value at the last element of row s is exactly that row's packed max
(+4096*s, which the tail's fraction-extraction discards). All values stay
exactly representable in f32 (q<=4095, 4096*13+4095+0.375 < 2^16 at ulp
2^-8).

ACT gathers the 14 per-row results of each tile (strided copy) into kmax
[128, 224]. Tail: three small custom DVE ops decode dpx/dpy exactly from
the fraction (magic-number rounds at 1 and 1/16), with target shifts
(tx+7.125, ty-0.8125) folded in on the host; ACT Squares+accumulates both
into one [128,1] partial per core; host sums 8x[128,1] / N.
"""
import numpy as np

B = 16384
NJ = 14
NPIX = 196
N_CORES = 8
ROWS_PER_TILE = 1792          # 128 partitions x 14 rows
K_PER_PART = 14
N_TILES = 16                  # (B/N_CORES)*NJ / ROWS_PER_TILE
ELEMS = K_PER_PART * NPIX     # 2744 per partition per tile
NCOLS = N_TILES * K_PER_PART  # 224

MAGIC23 = 12582912.0          # 1.5*2^23, ulp 1
M16 = 786432.0                # 1.5*2^19, ulp 1/16
CLAMP = 4095.0

_STATE = {}


def _register_ops():
    """Idempotently add our custom DVE ops to the concourse registry."""
    import concourse.dve_ops as dve_ops
    if "MSE7541_SCAN" in dve_ops._SUB_OPCODE_FOR_NAME:
        return {n: op for op in dve_ops.OPS
                for n in [op.name] if n.startswith("MSE7541_")}

    from concourse.dve_spec import (
        Spec, Src0, Src1, C0, C1, C2, relu, minn, scan, AluOp, lower,
        _has_src1 as has_src1,
    )
    from concourse.dve_uop import DveOpSpec

    # SCAN: running_max(((min(relu(h512), C1) + C0) - C0) + w8pg)
    v = minn(relu(Src0), C1)
    q = (v + C0) - C0
    scan_spec = Spec(
        body=scan(AluOp.MAX, q + Src1),
        reference=lambda in0, in1, s0, s1, imm2: np.maximum.accumulate(
            (np.float32(np.minimum(np.maximum(in0, 0), s1) + s0) - np.float32(s0))
            + in1, axis=-1).astype(np.float32),
    )

    # OPF: fraction extract fr = x - round(x) (round at ulp 1 via C0 magic)
    fr = Src0 - ((Src0 + C0) - C0)
    opf_spec = Spec(
        body=fr,
        reference=lambda in0, in1, s0, s1, imm2: (
            in0 - (np.float32(in0 + s0) - np.float32(s0))).astype(np.float32),
    )

    # OPX2: in0=fr, in1=txh (=tx+7.125): out = dpx
    #   q16 = round_{1/16}(fr + C0) via C2 magic; C0=0.46875, C1=16, C2=M16
    g = Src0 + C0
    q16 = (g + C2) - C2
    opx2_spec = Spec(
        body=(q16 * C1) - ((Src0 * C1) + Src1),
        reference=lambda in0, in1, s0, s1, imm2: (
            (np.float32(np.float32(in0 + s0) + imm2) - np.float32(imm2)) * s1
            - (in0 * s1 + in1)).astype(np.float32),
    )

    # OPY2: in0=fr, in1=tyh (=ty-0.8125): out = -dpy
    opy2_spec = Spec(
        body=q16 + Src1,
        reference=lambda in0, in1, s0, s1, imm2: (
            (np.float32(np.float32(in0 + s0) + imm2) - np.float32(imm2))
            + in1).astype(np.float32),
    )

    ops = {}
    for name, spec in [("MSE7541_SCAN", scan_spec), ("MSE7541_OPF", opf_spec),
                       ("MSE7541_OPX2", opx2_spec), ("MSE7541_OPY2", opy2_spec)]:
        row = dve_ops._CUSTOM_DVE_ROW_BASE + len(dve_ops.OPS)
        assert row < 0x20, "custom DVE row overflow"
        shas = {}
        for ver in ("v3", "v4"):
            try:
                uops = lower(spec, ver=ver)
                shas[ver] = DveOpSpec(
                    name=name, opcode=row, uops=uops,
                    rd1_en=has_src1(spec)).sha(ver)
            except Exception:
                pass
        op = dve_ops.DveOp(name, spec, subdim=False, uops_sha=shas)
        dve_ops.OPS.append(op)
        dve_ops.CUSTOM_DVE_SPECS[name] = spec
        dve_ops._SUB_OPCODE_FOR_NAME[name] = row
        ops[name] = op
    return ops


def _build():
    import concourse.bacc as bacc
    import concourse.mybir as mybir
    from concourse.tile import TileContext

    ops = _register_ops()
    F32 = mybir.dt.float32
    F16 = mybir.dt.float16
    AF = mybir.ActivationFunctionType

    rows = N_TILES * ROWS_PER_TILE

    nc = bacc.Bacc()
    h = nc.declare_dram_parameter("h", [rows, NPIX], F16, isOutput=False)
    wpg = nc.declare_dram_parameter("wpg", [128, ELEMS], F32, isOutput=False)
    txh = nc.declare_dram_parameter("txh", [128, NCOLS], F32, isOutput=False)
    tyh = nc.declare_dram_parameter("tyh", [128, NCOLS], F32, isOutput=False)
    out = nc.declare_dram_parameter("part", [128, 1], F32, isOutput=True)

    with TileContext(nc) as tc:
        with tc.tile_pool(name="hpool", bufs=6) as hpool, \
             tc.tile_pool(name="spool", bufs=4) as spool, \
             tc.tile_pool(name="consts", bufs=1) as cpool, \
             tc.tile_pool(name="acc", bufs=1) as accpool:
            wpgt = cpool.tile([128, ELEMS], F32, tag="wpgt")
            nc.sync.dma_start(wpgt[:], wpg[:])
            txt = cpool.tile([128, NCOLS], F32, tag="txt")
            nc.sync.dma_start(txt[:], txh[:])
            tyt = cpool.tile([128, NCOLS], F32, tag="tyt")
            nc.sync.dma_start(tyt[:], tyh[:])
            kmax = accpool.tile([128, NCOLS], F32, tag="kmax")

            for t in range(N_TILES):
                ht = hpool.tile([128, ELEMS], F16, tag="ht")
                # partition p owns DRAM rows t*1792 + p*14 .. +13 (contiguous)
                nc.sync.dma_start(
                    ht[:],
                    h[t * ROWS_PER_TILE:(t + 1) * ROWS_PER_TILE, :]
                    .rearrange("(p k) f -> p (k f)", p=128))
                so = spool.tile([128, ELEMS], F32, tag="so")
                nc.vector._custom_dve(
                    ops["MSE7541_SCAN"], out=so[:], in0=ht[:], in1=wpgt[:],
                    s0=MAGIC23, s1=CLAMP)
                # per-row maxes live at the last element of each 196-block
                last = (so.rearrange("p (k f) -> p k f", f=NPIX)
                        [:, :, NPIX - 1:NPIX])
                kslice = (kmax[:, t * K_PER_PART:(t + 1) * K_PER_PART]
                          .rearrange("p (k one) -> p k one", one=1))
                nc.scalar.activation(kslice, last, AF.Identity)

            fr = accpool.tile([128, NCOLS], F32, tag="fr")
            nc.vector._custom_dve(
                ops["MSE7541_OPF"], out=fr[:], in0=kmax[:], s0=MAGIC23)
            dxy = accpool.tile([128, 2 * NCOLS], F32, tag="dxy")
            nc.vector._custom_dve(
                ops["MSE7541_OPX2"], out=dxy[:, :NCOLS], in0=fr[:], in1=txt[:],
                s0=0.46875, s1=16.0, imm2=M16)
            nc.vector._custom_dve(
                ops["MSE7541_OPY2"], out=dxy[:, NCOLS:], in0=fr[:], in1=tyt[:],
                s0=0.46875, imm2=M16)
            sq = accpool.tile([128, 2 * NCOLS], F32, tag="sq")
            part_sb = accpool.tile([128, 1], F32, tag="part")
            nc.scalar.activation(sq[:], dxy[:], AF.Square,
                                 accum_out=part_sb[:])
            nc.sync.dma_start(out[:], part_sb[:])
    nc.finalize()
    return nc


def _w8pg_table() -> np.ndarray:
    i = np.arange(NPIX)
    y, x = i // 14, i % 14
    w8 = (13 - y) * 16 + (14 - x)                 # [1, 224]; ties -> first occ
    row = np.concatenate([
        (w8 - 128) / 256.0 + 4096.0 * s for s in range(K_PER_PART)])
    return np.broadcast_to(row.astype(np.float32), (128, ELEMS)).copy()


def _targets(t_shard: np.ndarray):
    bs = t_shard.shape[0]
    t2 = t_shard.reshape(bs, 28).astype(np.float64)
    tx = t2[:, :14].reshape(N_TILES, 128, K_PER_PART).transpose(1, 0, 2)
    ty = t2[:, 14:].reshape(N_TILES, 128, K_PER_PART).transpose(1, 0, 2)
    txh = (tx + 7.125).astype(np.float32).reshape(128, NCOLS)
    tyh = (ty - 0.8125).astype(np.float32).reshape(128, NCOLS)
    return np.ascontiguousarray(txh), np.ascontiguousarray(tyh)


def kernel(o: np.ndarray, h: np.ndarray, t: np.ndarray, v: np.ndarray,
           _trace: bool = False, _tmpdir: str | None = None) -> np.ndarray:
    from concourse.bass_utils import run_bass_kernel_spmd

    if "nc" not in _STATE:
        _STATE["nc"] = _build()
    nc = _STATE["nc"]

    h512 = (np.asarray(h, dtype=np.float32) * np.float32(512.0)).astype(np.float16)
    t = np.ascontiguousarray(np.asarray(t, dtype=np.float32))
    bs = B // N_CORES
    wpg = _w8pg_table()
    in_maps = []
    for c in range(N_CORES):
        h_shard = np.ascontiguousarray(
            h512[c * bs:(c + 1) * bs].reshape(bs * NJ, NPIX))
        txh, tyh = _targets(t[c * bs:(c + 1) * bs])
        in_maps.append({"h": h_shard, "wpg": wpg, "txh": txh, "tyh": tyh})

    res = run_bass_kernel_spmd(
        nc, in_maps, list(range(N_CORES)),
        trace=_trace, tmpdir=_tmpdir)
    _STATE["last_result"] = res
    total = np.float64(0.0)
    for c in range(N_CORES):
        total += np.asarray(res.results[c]["part"], dtype=np.float64).sum()
    n = np.float32(B * NJ)
    return np.float32(np.float32(total) / n)


# revision 5
# speedup vs baseline: 2.0334x; 1.0963x over previous
"""Trainium2 kernel for nn_MeanSquaredError2: MSE between argmax-decoded
heatmap coordinates and targets.

loss = sum_{b,j} [(px - tpx)^2 + (py - tpy)^2] / (B*NJ)
  where idx = argmax(h[b,j]), px = (idx%14)/16, py = (idx//14)/16 and
  (tpx, tpy) follow the reference's concat-then-reshape pairing of t.
Inputs o and v do not affect the result (USE_VISIBILITY=False).

Pure data parallel over 8 cores (2048 batches each). h is pre-scaled by 512
and converted to fp16 on the host (halves HBM traffic; quantization flips
the argmax on ~0.14% of rows, ~3e-4 relative loss error, tolerance 2e-2).

Per core, 16 tiles of [128 part x (14 rows x 196 pix)]. A single custom DVE
instruction per tile does the whole pack-and-max:
    k = ((min(relu(h512), 4095) + 1.5*2^23) - 1.5*2^23) + w8pg
    out = running_max(k)            (inclusive MAX-scan along the stream)
w8pg[s*196 + i] = (w8[i] - 128)/256 + 4096*s packs the reversed pixel index
w8 = (13-y)*16 + (14-x) into the fraction (first-occurrence tie-break) and a
per-row offset 4096*s that makes the running max resettable per row: the
scan value at the last element of row s is exactly that row's packed max
(+4096*s, which the tail's fraction-extraction discards). All values stay
exactly representable in f32 (q<=4095, 4096*13+4095+0.375 < 2^16 at ulp
2^-8).

ACT gathers the 14 per-row results of each tile (strided copy) into kmax
[128, 224]. Tail: three small custom DVE ops decode dpx/dpy exactly from
the fraction (magic-number rounds at 1 and 1/16), with target shifts
(tx+7.125, ty-0.8125) folded in on the host; ACT Squares+accumulates both
into one [128,1] partial per core; host sums 8x[128,1] / N.
"""
import numpy as np

B = 16384
NJ = 14
NPIX = 196
N_CORES = 8
ROWS_PER_TILE = 1792          # 128 partitions x 14 rows
K_PER_PART = 14
N_TILES = 16                  # (B/N_CORES)*NJ / ROWS_PER_TILE
ELEMS = K_PER_PART * NPIX     # 2744 per partition per tile
NCOLS = N_TILES * K_PER_PART  # 224

MAGIC23 = 12582912.0          # 1.5*2^23, ulp 1
M16 = 786432.0                # 1.5*2^19, ulp 1/16
CLAMP = 4095.0

_STATE = {}


def _register_ops():
    """Idempotently add our custom DVE ops to the concourse registry."""
    import concourse.dve_ops as dve_ops
    if "MSE7541_SCAN" in dve_ops._SUB_OPCODE_FOR_NAME:
        return {n: op for op in dve_ops.OPS
                for n in [op.name] if n.startswith("MSE7541_")}

    from concourse.dve_spec import (
        Spec, Src0, Src1, C0, C1, C2, relu, minn, scan, AluOp, lower,
        _has_src1 as has_src1,
    )
    from concourse.dve_uop import DveOpSpec

    # SCAN: running_max(((min(relu(h512), C1) + C0) - C0) + w8pg)
    v = minn(relu(Src0), C1)
    q = (v + C0) - C0
    scan_spec = Spec(
        body=scan(AluOp.MAX, q + Src1),
        reference=lambda in0, in1, s0, s1, imm2: np.maximum.accumulate(
            (np.float32(np.minimum(np.maximum(in0, 0), s1) + s0) - np.float32(s0))
            + in1, axis=-1).astype(np.float32),
    )

    # OPF: fraction extract fr = x - round(x) (round at ulp 1 via C0 magic)
    fr = Src0 - ((Src0 + C0) - C0)
    opf_spec = Spec(
        body=fr,
        reference=lambda in0, in1, s0, s1, imm2: (
            in0 - (np.float32(in0 + s0) - np.float32(s0))).astype(np.float32),
    )

    # OPX2: in0=fr, in1=txh (=tx+7.125): out = dpx
    #   q16 = round_{1/16}(fr + C0) via C2 magic; C0=0.46875, C1=16, C2=M16
    g = Src0 + C0
    q16 = (g + C2) - C2
    opx2_spec = Spec(
        body=(q16 * C1) - ((Src0 * C1) + Src1),
        reference=lambda in0, in1, s0, s1, imm2: (
            (np.float32(np.float32(in0 + s0) + imm2) - np.float32(imm2)) * s1
            - (in0 * s1 + in1)).astype(np.float32),
    )

    # OPY2: in0=fr, in1=tyh (=ty-0.8125): out = -dpy
    opy2_spec = Spec(
        body=q16 + Src1,
        reference=lambda in0, in1, s0, s1, imm2: (
            (np.float32(np.float32(in0 + s0) + imm2) - np.float32(imm2))
            + in1).astype(np.float32),
    )

    ops = {}
    for name, spec in [("MSE7541_SCAN", scan_spec), ("MSE7541_OPF", opf_spec),
                       ("MSE7541_OPX2", opx2_spec), ("MSE7541_OPY2", opy2_spec)]:
        row = dve_ops._CUSTOM_DVE_ROW_BASE + len(dve_ops.OPS)
        assert row < 0x20, "custom DVE row overflow"
        shas = {}
        for ver in ("v3", "v4"):
            try:
                uops = lower(spec, ver=ver)
                shas[ver] = DveOpSpec(
                    name=name, opcode=row, uops=uops,
                    rd1_en=has_src1(spec)).sha(ver)
            except Exception:
                pass
        op = dve_ops.DveOp(name, spec, subdim=False, uops_sha=shas)
        dve_ops.OPS.append(op)
        dve_ops.CUSTOM_DVE_SPECS[name] = spec
        dve_ops._SUB_OPCODE_FOR_NAME[name] = row
        ops[name] = op
    return ops


def _build():
    import concourse.bacc as bacc
    import concourse.mybir as mybir
    from concourse.tile import TileContext

    ops = _register_ops()
    F32 = mybir.dt.float32
    F16 = mybir.dt.float16
    AF = mybir.ActivationFunctionType

    rows = N_TILES * ROWS_PER_TILE

    nc = bacc.Bacc()
    h = nc.declare_dram_parameter("h", [rows, NPIX], F16, isOutput=False)
    wpg = nc.declare_dram_parameter("wpg", [128, ELEMS], F32, isOutput=False)
    txh = nc.declare_dram_parameter("txh", [128, NCOLS], F32, isOutput=False)
    tyh = nc.declare_dram_parameter("tyh", [128, NCOLS], F32, isOutput=False)
    out = nc.declare_dram_parameter("part", [1, 1], F32, isOutput=True)

    with TileContext(nc) as tc:
        with tc.tile_pool(name="hpool", bufs=6) as hpool, \
             tc.tile_pool(name="spool", bufs=4) as spool, \
             tc.tile_pool(name="consts", bufs=1) as cpool, \
             tc.tile_pool(name="acc", bufs=1) as accpool:
            # queue order matters: tile 0's h + the wpg table gate the first
            # scan; everything else (targets, later tiles) comes after.
            ht0 = hpool.tile([128, ELEMS], F16, tag="ht")
            nc.sync.dma_start(
                ht0[:], h[0:ROWS_PER_TILE, :].rearrange("(p k) f -> p (k f)", p=128))
            wpgt = cpool.tile([128, ELEMS], F32, tag="wpgt")
            nc.sync.dma_start(wpgt[:], wpg[:])
            txt = cpool.tile([128, NCOLS], F32, tag="txt")
            tyt = cpool.tile([128, NCOLS], F32, tag="tyt")
            kmax = accpool.tile([128, NCOLS], F32, tag="kmax")

            for t in range(N_TILES):
                if t == 0:
                    ht = ht0
                else:
                    ht = hpool.tile([128, ELEMS], F16, tag="ht")
                    # partition p owns DRAM rows t*1792 + p*14 .. +13 (contig)
                    nc.sync.dma_start(
                        ht[:],
                        h[t * ROWS_PER_TILE:(t + 1) * ROWS_PER_TILE, :]
                        .rearrange("(p k) f -> p (k f)", p=128))
                if t == 2:
                    nc.sync.dma_start(txt[:], txh[:])
                    nc.sync.dma_start(tyt[:], tyh[:])
                so = spool.tile([128, ELEMS], F32, tag="so")
                nc.vector._custom_dve(
                    ops["MSE7541_SCAN"], out=so[:], in0=ht[:], in1=wpgt[:],
                    s0=MAGIC23, s1=CLAMP)
                # per-row maxes live at the last element of each 196-block
                last = (so.rearrange("p (k f) -> p k f", f=NPIX)
                        [:, :, NPIX - 1:NPIX])
                kslice = (kmax[:, t * K_PER_PART:(t + 1) * K_PER_PART]
                          .rearrange("p (k one) -> p k one", one=1))
                nc.scalar.activation(kslice, last, AF.Identity)

            fr = accpool.tile([128, NCOLS], F32, tag="fr")
            nc.vector._custom_dve(
                ops["MSE7541_OPF"], out=fr[:], in0=kmax[:], s0=MAGIC23)
            dxy = accpool.tile([128, 2 * NCOLS], F32, tag="dxy")
            nc.vector._custom_dve(
                ops["MSE7541_OPX2"], out=dxy[:, :NCOLS], in0=fr[:], in1=txt[:],
                s0=0.46875, s1=16.0, imm2=M16)
            nc.vector._custom_dve(
                ops["MSE7541_OPY2"], out=dxy[:, NCOLS:], in0=fr[:], in1=tyt[:],
                s0=0.46875, imm2=M16)
            sq = accpool.tile([128, 2 * NCOLS], F32, tag="sq")
            part_sb = accpool.tile([128, 1], F32, tag="part")
            nc.scalar.activation(sq[:], dxy[:], AF.Square,
                                 accum_out=part_sb[:])
            # cross-partition sum on Pool so the output DMA is a single
            # 4-byte descriptor (a [128,1] DMA pays ~9us of per-engine
            # completion-semaphore latency at the final barrier)
            import concourse.bass_isa as bass_isa
            red = accpool.tile([128, 1], F32, tag="red")
            nc.gpsimd.partition_all_reduce(
                red[:], part_sb[:], channels=128,
                reduce_op=bass_isa.ReduceOp.add)
            nc.sync.dma_start(out[:], red[0:1, :])
    nc.finalize()
    return nc


def _w8pg_table() -> np.ndarray:
    i = np.arange(NPIX)
    y, x = i // 14, i % 14
    w8 = (13 - y) * 16 + (14 - x)                 # [1, 224]; ties -> first occ
    row = np.concatenate([
        (w8 - 128) / 256.0 + 4096.0 * s for s in range(K_PER_PART)])
    return np.broadcast_to(row.astype(np.float32), (128, ELEMS)).copy()


def _targets(t_shard: np.ndarray):
    bs = t_shard.shape[0]
    t2 = t_shard.reshape(bs, 28).astype(np.float64)
    tx = t2[:, :14].reshape(N_TILES, 128, K_PER_PART).transpose(1, 0, 2)
    ty = t2[:, 14:].reshape(N_TILES, 128, K_PER_PART).transpose(1, 0, 2)
    txh = (tx + 7.125).astype(np.float32).reshape(128, NCOLS)
    tyh = (ty - 0.8125).astype(np.float32).reshape(128, NCOLS)
    return np.ascontiguousarray(txh), np.ascontiguousarray(tyh)


def kernel(o: np.ndarray, h: np.ndarray, t: np.ndarray, v: np.ndarray,
           _trace: bool = False, _tmpdir: str | None = None) -> np.ndarray:
    from concourse.bass_utils import run_bass_kernel_spmd

    if "nc" not in _STATE:
        _STATE["nc"] = _build()
    nc = _STATE["nc"]

    h512 = (np.asarray(h, dtype=np.float32) * np.float32(512.0)).astype(np.float16)
    t = np.ascontiguousarray(np.asarray(t, dtype=np.float32))
    bs = B // N_CORES
    wpg = _w8pg_table()
    in_maps = []
    for c in range(N_CORES):
        h_shard = np.ascontiguousarray(
            h512[c * bs:(c + 1) * bs].reshape(bs * NJ, NPIX))
        txh, tyh = _targets(t[c * bs:(c + 1) * bs])
        in_maps.append({"h": h_shard, "wpg": wpg, "txh": txh, "tyh": tyh})

    res = run_bass_kernel_spmd(
        nc, in_maps, list(range(N_CORES)),
        trace=_trace, tmpdir=_tmpdir)
    _STATE["last_result"] = res
    total = np.float64(0.0)
    for c in range(N_CORES):
        total += float(np.asarray(res.results[c]["part"]).reshape(-1)[0])
    n = np.float32(B * NJ)
    return np.float32(np.float32(total) / n)


# revision 22
# speedup vs baseline: 2.1415x; 1.0532x over previous
"""Trainium2 kernel for nn_MeanSquaredError2: MSE between argmax-decoded
heatmap coordinates and targets.

loss = sum_{b,j} [(px - tpx)^2 + (py - tpy)^2] / (B*NJ)
  where idx = argmax(h[b,j]), px = (idx%14)/16, py = (idx//14)/16 and
  (tpx, tpy) follow the reference's concat-then-reshape pairing of t.
Inputs o and v do not affect the result (USE_VISIBILITY=False).

Pure data parallel over 8 cores (2048 batches each). h is pre-scaled by 512
and converted to fp16 on the host (halves HBM traffic; quantization flips
the argmax on ~0.14% of rows, ~3e-4 relative loss error, tolerance 2e-2).

Per core, 16 tiles of [128 part x (14 rows x 196 pix)]. A single custom DVE
instruction per tile does the whole pack-and-max:
    k = ((min(relu(h512), 4095) + 1.5*2^23) - 1.5*2^23) + w8pg
    out = running_max(k)            (inclusive MAX-scan along the stream)
w8pg[s*196 + i] = (w8[i] - 128)/256 + 4096*s packs the reversed pixel index
w8 = (13-y)*16 + (14-x) into the fraction (first-occurrence tie-break) and a
per-row offset 4096*s that makes the running max resettable per row: the
scan value at the last element of row s is exactly that row's packed max
(+4096*s, which the tail's fraction-extraction discards). All values stay
exactly representable in f32 (q<=4095, 4096*13+4095+0.375 < 2^16 at ulp
2^-8).

ACT gathers the 14 per-row results of each tile (strided copy) into kmax
[128, 224]. Tail: three small custom DVE ops decode dpx/dpy exactly from
the fraction (magic-number rounds at 1 and 1/16), with target shifts
(tx+7.125, ty-0.8125) folded in on the host; ACT Squares+accumulates both
into one [128,1] partial per core; host sums 8x[128,1] / N.
"""
import numpy as np

B = 16384
NJ = 14
NPIX = 196
N_CORES = 8
ROWS_PER_TILE = 1792          # 128 partitions x 14 rows
K_PER_PART = 14
N_TILES = 16                  # (B/N_CORES)*NJ / ROWS_PER_TILE
ELEMS = K_PER_PART * NPIX     # 2744 per partition per tile
NCOLS = N_TILES * K_PER_PART  # 224

MAGIC23 = 12582912.0          # 1.5*2^23, ulp 1
M16 = 786432.0                # 1.5*2^19, ulp 1/16
CLAMP = 4095.0

_STATE = {}


def _register_ops():
    """Idempotently add our custom DVE ops to the concourse registry."""
    import concourse.dve_ops as dve_ops
    if "MSE7541_SCAN" in dve_ops._SUB_OPCODE_FOR_NAME:
        return {n: op for op in dve_ops.OPS
                for n in [op.name] if n.startswith("MSE7541_")}

    from concourse.dve_spec import (
        Spec, Src0, Src1, C0, C1, C2, relu, minn, scan, AluOp, lower,
        _has_src1 as has_src1,
    )
    from concourse.dve_uop import DveOpSpec

    # SCAN: running_max(((min(relu(h512), C1) + C0) - C0) + w8pg)
    v = minn(relu(Src0), C1)
    q = (v + C0) - C0
    scan_spec = Spec(
        body=scan(AluOp.MAX, q + Src1),
        reference=lambda in0, in1, s0, s1, imm2: np.maximum.accumulate(
            (np.float32(np.minimum(np.maximum(in0, 0), s1) + s0) - np.float32(s0))
            + in1, axis=-1).astype(np.float32),
    )

    # OPF: fraction extract fr = x - round(x) (round at ulp 1 via C0 magic)
    fr = Src0 - ((Src0 + C0) - C0)
    opf_spec = Spec(
        body=fr,
        reference=lambda in0, in1, s0, s1, imm2: (
            in0 - (np.float32(in0 + s0) - np.float32(s0))).astype(np.float32),
    )

    # OPX2: in0=fr, in1=txh (=tx+7.125): out = dpx
    #   q16 = round_{1/16}(fr + C0) via C2 magic; C0=0.46875, C1=16, C2=M16
    g = Src0 + C0
    q16 = (g + C2) - C2
    opx2_spec = Spec(
        body=(q16 * C1) - ((Src0 * C1) + Src1),
        reference=lambda in0, in1, s0, s1, imm2: (
            (np.float32(np.float32(in0 + s0) + imm2) - np.float32(imm2)) * s1
            - (in0 * s1 + in1)).astype(np.float32),
    )

    # OPY2: in0=fr, in1=tyh (=ty-0.8125): out = -dpy
    opy2_spec = Spec(
        body=q16 + Src1,
        reference=lambda in0, in1, s0, s1, imm2: (
            (np.float32(np.float32(in0 + s0) + imm2) - np.float32(imm2))
            + in1).astype(np.float32),
    )

    ops = {}
    for name, spec in [("MSE7541_SCAN", scan_spec), ("MSE7541_OPF", opf_spec),
                       ("MSE7541_OPX2", opx2_spec), ("MSE7541_OPY2", opy2_spec)]:
        row = dve_ops._CUSTOM_DVE_ROW_BASE + len(dve_ops.OPS)
        assert row < 0x20, "custom DVE row overflow"
        shas = {}
        for ver in ("v3", "v4"):
            try:
                uops = lower(spec, ver=ver)
                shas[ver] = DveOpSpec(
                    name=name, opcode=row, uops=uops,
                    rd1_en=has_src1(spec)).sha(ver)
            except Exception:
                pass
        op = dve_ops.DveOp(name, spec, subdim=False, uops_sha=shas)
        dve_ops.OPS.append(op)
        dve_ops.CUSTOM_DVE_SPECS[name] = spec
        dve_ops._SUB_OPCODE_FOR_NAME[name] = row
        ops[name] = op
    return ops


import os
# Pool/ACT offload is dead on this toolchain: Pool TensorTensor supports
# add/mult but NOT max (ISA check fails at codegen), so per-row max only
# runs on DVE. Keep the hook for experiments; default off.
OFFLOAD = tuple(int(x) for x in os.environ.get("MSE_OFFLOAD", "").split(",")
                if x != "")
TBL_MODE = os.environ.get("MSE_TBL", "stt")  # stt | act
# disjoint-halving schedule for 196 -> 1 per row: (out_w, in_lo, in_hi);
# out[0:out_w] = max(in[0:out_w], in[in_lo:in_hi]); col 48 merged at the end
TREE = [(98, 98, 196), (49, 49, 98), (24, 24, 48), (12, 12, 24), (6, 6, 12),
        (3, 3, 6), (1, 1, 2), (1, 2, 3), (1, 48, 49)]


def _build():
    import concourse.bacc as bacc
    import concourse.mybir as mybir
    from concourse.tile import TileContext

    ops = _register_ops()
    F32 = mybir.dt.float32
    F16 = mybir.dt.float16
    AF = mybir.ActivationFunctionType
    A = mybir.AluOpType

    rows = N_TILES * ROWS_PER_TILE

    nc = bacc.Bacc()
    h = nc.declare_dram_parameter("h", [rows, NPIX], F16, isOutput=False)
    # w8f[196] | pgc[14] | mgb[2] packed into one param -> one DMA issue
    cst = nc.declare_dram_parameter("cst", [128, NPIX + K_PER_PART + 2], F32,
                                    isOutput=False)
    txh = nc.declare_dram_parameter("txh", [128, NCOLS], F32, isOutput=False)
    tyh = nc.declare_dram_parameter("tyh", [128, NCOLS], F32, isOutput=False)
    out = nc.declare_dram_parameter("part", [1, 1], F32, isOutput=True)

    with TileContext(nc) as tc:
        with tc.tile_pool(name="hpool", bufs=6) as hpool, \
             tc.tile_pool(name="spool", bufs=4) as spool, \
             tc.tile_pool(name="bpool", bufs=2) as bpool, \
             tc.tile_pool(name="consts", bufs=1) as cpool, \
             tc.tile_pool(name="acc", bufs=1) as accpool:
            # tiny consts first, then the h tiles in order; the wpg table is
            # built on-chip (one DVE stt) instead of a 1.4MB DMA, so scan 0
            # starts ~4us earlier.
            cstt = cpool.tile([128, NPIX + K_PER_PART + 2], F32, tag="cstt")
            nc.sync.dma_start(cstt[:], cst[:])
            w8ft = cstt[:, 0:NPIX]
            pgct = cstt[:, NPIX:NPIX + K_PER_PART]
            mgt = cstt[:, NPIX + K_PER_PART:]
            w8f_b = (w8ft.rearrange("p (o f) -> p o f", o=1)
                     .broadcast_to([128, K_PER_PART, NPIX]))
            pgc_b = (pgct.rearrange("p (k o) -> p k o", o=1)
                     .broadcast_to([128, K_PER_PART, NPIX]))
            wpgt = cpool.tile([128, ELEMS], F32, tag="wpgt")
            if TBL_MODE == "stt":
                nc.vector.scalar_tensor_tensor(
                    wpgt.rearrange("p (k f) -> p k f", f=NPIX),
                    w8f_b, 1.0, pgc_b, op0=A.mult, op1=A.add)
            else:
                # 14 small ACT passes: page s = w8f + 4096*s (bias AP per page)
                for s in range(K_PER_PART):
                    nc.scalar.activation(
                        wpgt[:, s * NPIX:(s + 1) * NPIX], w8ft[:],
                        AF.Identity, bias=pgct[:, s:s + 1])
            txt = cpool.tile([128, NCOLS], F32, tag="txt")
            tyt = cpool.tile([128, NCOLS], F32, tag="tyt")
            kmax = accpool.tile([128, NCOLS], F32, tag="kmax")

            for t in range(N_TILES):
                ht = hpool.tile([128, ELEMS], F16, tag="ht")
                # partition p owns DRAM rows t*1792 + p*14 .. +13 (contig)
                nc.sync.dma_start(
                    ht[:],
                    h[t * ROWS_PER_TILE:(t + 1) * ROWS_PER_TILE, :]
                    .rearrange("(p k) f -> p (k f)", p=128))
                if t == 2:
                    nc.sync.dma_start(txt[:], txh[:])
                    nc.sync.dma_start(tyt[:], tyh[:])
                kslice = (kmax[:, t * K_PER_PART:(t + 1) * K_PER_PART]
                          .rearrange("p (k one) -> p k one", one=1))
                if t in OFFLOAD:
                    # ACT pack: q = round(h512) via magic add/sub (2 passes;
                    # the sub must happen before adding the index fraction,
                    # or f32 ulp-1 at the magic destroys it)
                    pk = bpool.tile([128, ELEMS], F32, tag="pk")
                    nc.scalar.activation(pk[:], ht[:], AF.Identity,
                                         bias=mgt[:, 0:1])
                    nc.scalar.activation(pk[:], pk[:], AF.Identity,
                                         bias=mgt[:, 1:2])
                    pk3 = pk.rearrange("p (k f) -> p k f", f=NPIX)
                    nc.gpsimd.tensor_tensor(pk3, pk3, w8f_b, op=A.add)
                    for w_out, in_lo, in_hi in TREE:
                        nc.gpsimd.tensor_tensor(
                            pk3[:, :, 0:w_out], pk3[:, :, 0:w_out],
                            pk3[:, :, in_lo:in_hi], op=A.max)
                    nc.scalar.activation(kslice, pk3[:, :, 0:1], AF.Identity)
                    continue
                so = spool.tile([128, ELEMS], F32, tag="so")
                nc.vector._custom_dve(
                    ops["MSE7541_SCAN"], out=so[:], in0=ht[:], in1=wpgt[:],
                    s0=MAGIC23, s1=CLAMP)
                # per-row maxes live at the last element of each 196-block
                last = (so.rearrange("p (k f) -> p k f", f=NPIX)
                        [:, :, NPIX - 1:NPIX])
                nc.scalar.activation(kslice, last, AF.Identity)

            fr = accpool.tile([128, NCOLS], F32, tag="fr")
            nc.vector._custom_dve(
                ops["MSE7541_OPF"], out=fr[:], in0=kmax[:], s0=MAGIC23)
            dxy = accpool.tile([128, 2 * NCOLS], F32, tag="dxy")
            nc.vector._custom_dve(
                ops["MSE7541_OPX2"], out=dxy[:, :NCOLS], in0=fr[:], in1=txt[:],
                s0=0.46875, s1=16.0, imm2=M16)
            nc.vector._custom_dve(
                ops["MSE7541_OPY2"], out=dxy[:, NCOLS:], in0=fr[:], in1=tyt[:],
                s0=0.46875, imm2=M16)
            sq = accpool.tile([128, 2 * NCOLS], F32, tag="sq")
            part_sb = accpool.tile([128, 1], F32, tag="part")
            nc.scalar.activation(sq[:], dxy[:], AF.Square,
                                 accum_out=part_sb[:])
            # cross-partition sum on Pool so the output DMA is a single
            # 4-byte descriptor (a [128,1] DMA pays ~9us of per-engine
            # completion-semaphore latency at the final barrier)
            import concourse.bass_isa as bass_isa
            red = accpool.tile([128, 1], F32, tag="red")
            nc.gpsimd.partition_all_reduce(
                red[:], part_sb[:], channels=128,
                reduce_op=bass_isa.ReduceOp.add)
            nc.sync.dma_start(out[:], red[0:1, :])
    nc.finalize()
    return nc


def _w8f_table() -> np.ndarray:
    i = np.arange(NPIX)
    y, x = i // 14, i % 14
    w8 = (13 - y) * 16 + (14 - x)                 # [1, 224]; ties -> first occ
    row = ((w8 - 128) / 256.0).astype(np.float32)
    return np.broadcast_to(row, (128, NPIX)).copy()


def _pgc_table() -> np.ndarray:
    row = (4096.0 * np.arange(K_PER_PART)).astype(np.float32)
    return np.broadcast_to(row, (128, K_PER_PART)).copy()


def _targets(t_shard: np.ndarray):
    bs = t_shard.shape[0]
    t2 = t_shard.reshape(bs, 28).astype(np.float64)
    tx = t2[:, :14].reshape(N_TILES, 128, K_PER_PART).transpose(1, 0, 2)
    ty = t2[:, 14:].reshape(N_TILES, 128, K_PER_PART).transpose(1, 0, 2)
    txh = (tx + 7.125).astype(np.float32).reshape(128, NCOLS)
    tyh = (ty - 0.8125).astype(np.float32).reshape(128, NCOLS)
    return np.ascontiguousarray(txh), np.ascontiguousarray(tyh)


def kernel(o: np.ndarray, h: np.ndarray, t: np.ndarray, v: np.ndarray,
           _trace: bool = False, _tmpdir: str | None = None) -> np.ndarray:
    from concourse.bass_utils import run_bass_kernel_spmd

    if "nc" not in _STATE:
        _STATE["nc"] = _build()
    nc = _STATE["nc"]

    h512 = (np.asarray(h, dtype=np.float32) * np.float32(512.0)).astype(np.float16)
    t = np.ascontiguousarray(np.asarray(t, dtype=np.float32))
    bs = B // N_CORES
    mgb = np.broadcast_to(
        np.array([MAGIC23, -MAGIC23], np.float32), (128, 2))
    cst = np.ascontiguousarray(
        np.concatenate([_w8f_table(), _pgc_table(), mgb], axis=1))
    in_maps = []
    for c in range(N_CORES):
        h_shard = np.ascontiguousarray(
            h512[c * bs:(c + 1) * bs].reshape(bs * NJ, NPIX))
        txh, tyh = _targets(t[c * bs:(c + 1) * bs])
        in_maps.append({"h": h_shard, "cst": cst, "txh": txh, "tyh": tyh})

    res = run_bass_kernel_spmd(
        nc, in_maps, list(range(N_CORES)),
        trace=_trace, tmpdir=_tmpdir)
    _STATE["last_result"] = res
    total = np.float64(0.0)
    for c in range(N_CORES):
        total += float(np.asarray(res.results[c]["part"]).reshape(-1)[0])
    n = np.float32(B * NJ)
    return np.float32(np.float32(total) / n)
